# revision 13
# baseline (speedup 1.0000x reference)
"""Trainium2 Bass kernel for dense layer: out = inputs @ kernel + bias.

Shapes (hardcoded): inputs [16384, 768] f32, kernel [768, 768] f32,
bias [768] f32 -> out [16384, 768] f32.

Strategy: data-parallel over 8 NeuronCores; each core gets a contiguous
2048-row slice of `inputs`, kernel/bias replicated, no collectives.

v2 (bf16 datapath): x and W are rounded to bf16 on the host and y is
returned as bf16 (converted back to f32 on host). Rel err ~3e-3, well
under the 2e-2 gate, and it buys:
  - half the DMA bytes (7.5 MB/core vs 15), so the kernel is purely
    PE-bound and all 16 x tiles are prefetched upfront;
  - 1.0 cyc/row PE transposes (vs 1.5 f32r) and one fused [128,768]
    PSUM->SBUF eviction per tile (vs two);
  - matmul streams at the same 1 col/cycle as f32r, accumulated f32.
Engine layout:
  - sync (SP hwdge): all x-tile loads + W in 3 chunk-group DMAs,
    issued back-to-back upfront in arrival-need order.
  - scalar (Act hwdge): bias load, then all y writebacks -- a second,
    parallel DMA-issue path so y stores never queue behind x loads
    (in v1 a single sync ring serialized 51 DMA issues at ~630ns and
    starved the last tiles).
  - vector: builds the transpose identity on-chip (memset +
    affine_select; no DMA), one CAST eviction + two bias-add
    evictions per tile.
  - gpsimd: bias partition-broadcast only.
  - tensor: 6 transposes + 12 accumulating matmuls per tile; dummy
    identity transposes pad DMA-bound startup stalls so the PE
    p-state/HAM window keeps the clock up.
PSUM: tp 2 + warm 1 + p0 3 + p1 2 = 8 banks.
"""

import sys

for _p in ("/opt/trn_rl_repo", "/root/.axon_site/_ro/trn_rl_repo"):
    if _p not in sys.path:
        sys.path.insert(0, _p)

import numpy as np

B, IN, UNITS = 16384, 768, 768
N_CORES = 8
B_CORE = B // N_CORES          # 2048 rows per core
P = 128
KC = IN // P                   # 6 contraction chunks
NT = B_CORE // P               # 16 row tiles per core
N0, N1 = 512, UNITS - 512      # PSUM bank split of the 768 output cols
NG = 3                         # W arrives in 3 chunk-group DMAs of 2

_cache = {}


def _build_nc():
    import concourse.mybir as mybir
    import concourse.tile as tile
    from concourse import bacc

    f32 = mybir.dt.float32
    bf16 = mybir.dt.bfloat16

    nc = bacc.Bacc()
    x = nc.dram_tensor("x", [B_CORE, IN], bf16, kind="ExternalInput")
    w = nc.dram_tensor("w", [IN, UNITS], bf16, kind="ExternalInput")
    b = nc.dram_tensor("b", [UNITS], f32, kind="ExternalInput")
    y = nc.dram_tensor("y", [B_CORE, UNITS], bf16, kind="ExternalOutput")

    x_v = x.rearrange("(g p) i -> p g i", p=P)   # row tile g, partition p
    y_v = y.rearrange("(g p) u -> p g u", p=P)
    w_v = w.rearrange("(c p) u -> p c u", p=P)   # k-chunk c, partition p

    with tile.TileContext(nc) as tc:
        with (
            tc.tile_pool(name="const", bufs=1) as const,
            tc.tile_pool(name="xin", bufs=NT) as xin,
            tc.tile_pool(name="xt", bufs=4) as xt,
            tc.tile_pool(name="yout", bufs=3) as yout,
            tc.tile_pool(name="tp", bufs=2, space="PSUM") as tp_pool,
            tc.tile_pool(name="pa0", bufs=3, space="PSUM") as pa0_pool,
            tc.tile_pool(name="pa1", bufs=2, space="PSUM") as pa1_pool,
        ):
            # ---- DMA issue: sync = x tiles + W, in arrival-need order ----
            x_bufs = {}

            def dma_x(t):
                xb = xin.tile([P, IN], bf16, tag="x_buf")
                x_bufs[t] = xb
                nc.sync.dma_start(out=xb[:], in_=x_v[:, t, :])

            w_r = const.tile([P, KC, UNITS], bf16, tag="w_r")

            def dma_w(c):
                nc.scalar.dma_start(
                    out=w_r[:, c : c + 1, :], in_=w_v[:, c : c + 1, :]
                )

            # two parallel inbound streams from t=0 so both hwdge rings
            # pull bytes through the DMA-fabric ramp: all x tiles on the
            # sync ring, per-chunk W pieces + bias on the scalar ring
            # (y stores follow on scalar once tiles complete)
            for t in range(NT):
                dma_x(t)
            for c in range(KC):
                dma_w(c)
            bias1 = const.tile([1, UNITS], f32, tag="bias1")
            nc.scalar.dma_start(out=bias1[:], in_=b[None, :])

            # ---- transpose identity built on gpsimd (no DMA): emitted
            # before the bias broadcast so it isn't gated on the bias DMA ----
            ones = const.tile([P, P], bf16, tag="ones")
            nc.gpsimd.memset(ones[:], 1.0)
            ident = const.tile([P, P], bf16, tag="ident")
            nc.gpsimd.affine_select(
                ident[:],
                ones[:],
                pattern=[[1, P]],
                compare_op=mybir.AluOpType.is_equal,
                fill=0.0,
                base=0,
                channel_multiplier=-1,
            )

            bias_b = const.tile([P, UNITS], f32, tag="bias_b")
            nc.gpsimd.partition_broadcast(bias_b[:], bias1[:1, :])

            warm = tp_pool.tile([P, P], bf16, tag="warm", bufs=1)

            def pad(n):
                # dummy PE work: absorbs DMA-bound startup stalls so the
                # PE p-state ramp / HAM window never drops the clock
                for _ in range(n):
                    nc.tensor.transpose(warm[:], ident[:], ident[:])

            def emit_transposes(t):
                """PE-transpose tile t's 6 chunks into one PSUM bank and
                evict with a single fused DVE copy."""
                xn = x_bufs.pop(t)
                tp = tp_pool.tile([P, KC * P], bf16, tag="tp")
                for c in range(KC):
                    nc.tensor.transpose(
                        tp[:, c * P : (c + 1) * P],
                        xn[:, c * P : (c + 1) * P],
                        ident[:],
                    )
                xt_r = xt.tile([P, KC, P], bf16, tag="xt_r")
                nc.vector.tensor_copy(
                    xt_r[:].rearrange("p c b -> p (c b)"), tp[:]
                )
                return xt_r

            def open_accum():
                p0 = pa0_pool.tile([P, N0], f32, tag="p0")
                p1 = pa1_pool.tile([P, N1], f32, tag="p1")
                return p0, p1

            def accum_chunk(xt_r, p0, p1, c):
                lhsT = xt_r[:, c, :]                   # [128 i, 128 b]
                nc.tensor.matmul(
                    p0[:], lhsT, w_r[:, c, 0:N0],
                    start=(c == 0), stop=(c == KC - 1),
                )
                nc.tensor.matmul(
                    p1[:], lhsT, w_r[:, c, N0:UNITS],
                    start=(c == 0), stop=(c == KC - 1),
                )

            def accum_group(xt_r, p0, p1, g):
                for c in (2 * g, 2 * g + 1):
                    accum_chunk(xt_r, p0, p1, c)

            def evict(t, p0, p1):
                # bias-add eviction, per PSUM half; half 1 first so the
                # 2-buf pa1 pool frees a slot earlier (the p1-start matmul
                # two tiles later WAR-waits on this read)
                y_buf = yout.tile([P, UNITS], bf16, tag="y_buf")
                with nc.allow_low_precision(reason="bf16 output rounding"):
                    nc.vector.tensor_add(
                        y_buf[:, N0:UNITS], p1[:], bias_b[:, N0:UNITS]
                    )
                    nc.scalar.dma_start(
                        out=y_v[:, t, N0:UNITS], in_=y_buf[:, N0:UNITS]
                    )
                    nc.vector.tensor_add(
                        y_buf[:, 0:N0], p0[:], bias_b[:, 0:N0]
                    )
                    nc.scalar.dma_start(
                        out=y_v[:, t, 0:N0], in_=y_buf[:, 0:N0]
                    )

            # ---- startup: DMA arrival order is x0, W0, W1, x1, W2, W3,
            # x2, W4, W5, x3, ... — PE work is emitted in exactly that
            # order (PE executes in program order, so a stalled op would
            # block ready work behind it), with pads absorbing the
            # DMA-ramp stalls so the PE p-state keeps climbing ----
            pad(10)
            xts = {0: emit_transposes(0)}
            pad(3)
            pa = {0: open_accum(), 1: open_accum()}
            accum_chunk(xts[0], *pa[0], 0)   # gated on W0
            accum_chunk(xts[0], *pa[0], 1)   # W1
            pad(2)
            xts[1] = emit_transposes(1)      # x1
            accum_chunk(xts[1], *pa[1], 0)
            accum_chunk(xts[1], *pa[1], 1)
            accum_chunk(xts[0], *pa[0], 2)   # W2
            accum_chunk(xts[0], *pa[0], 3)   # W3
            accum_chunk(xts[1], *pa[1], 2)
            accum_chunk(xts[1], *pa[1], 3)
            xts[2] = emit_transposes(2)      # x2
            accum_chunk(xts[0], *pa[0], 4)   # W4
            accum_chunk(xts[0], *pa[0], 5)   # W5
            evict(0, *pa.pop(0))
            accum_chunk(xts[1], *pa[1], 4)
            accum_chunk(xts[1], *pa[1], 5)
            evict(1, *pa.pop(1))
            xts[3] = emit_transposes(3)      # x3

            # ---- steady state: transpose(t+2) right before accum(t), so
            # each tile's CAST eviction lands well before the PE needs it ----
            for t in range(2, NT - 1):
                if t + 2 < NT:
                    xts[t + 2] = emit_transposes(t + 2)
                p0, p1 = open_accum()
                xt_r = xts.pop(t)
                for g in range(NG):
                    accum_group(xt_r, p0, p1, g)
                evict(t, p0, p1)

            # last tile: run all 6 p1 chunks first, then all 6 p0 chunks,
            # so the half-1 eviction + writeback overlap the p0 matmuls
            # and the drain tail shortens
            t = NT - 1
            p0, p1 = open_accum()
            xt_r = xts.pop(t)
            y_buf = yout.tile([P, UNITS], bf16, tag="y_buf")
            for c in range(KC):
                nc.tensor.matmul(
                    p1[:], xt_r[:, c, :], w_r[:, c, N0:UNITS],
                    start=(c == 0), stop=(c == KC - 1),
                )
            with nc.allow_low_precision(reason="bf16 output rounding"):
                nc.vector.tensor_add(
                    y_buf[:, N0:UNITS], p1[:], bias_b[:, N0:UNITS]
                )
            nc.scalar.dma_start(out=y_v[:, t, N0:UNITS], in_=y_buf[:, N0:UNITS])
            for c in range(KC):
                nc.tensor.matmul(
                    p0[:], xt_r[:, c, :], w_r[:, c, 0:N0],
                    start=(c == 0), stop=(c == KC - 1),
                )
            # quarter-split final eviction, issues alternating between the
            # idle sync ring and scalar, to shorten the TT->issue->xfer
            # drain chain after the last matmul
            with nc.allow_low_precision(reason="bf16 output rounding"):
                nc.vector.tensor_add(
                    y_buf[:, 0:256], p0[:, 0:256], bias_b[:, 0:256]
                )
                nc.sync.dma_start(out=y_v[:, t, 0:256], in_=y_buf[:, 0:256])
                nc.vector.tensor_add(
                    y_buf[:, 256:N0], p0[:, 256:N0], bias_b[:, 256:N0]
                )
                nc.scalar.dma_start(
                    out=y_v[:, t, 256:N0], in_=y_buf[:, 256:N0]
                )

    nc.finalize()
    return nc


def _run(inputs, kernel, bias, trace=False, **kw):
    from concourse.bass_utils import run_bass_kernel_spmd
    import ml_dtypes

    if "nc" not in _cache:
        _cache["nc"] = _build_nc()
    nc = _cache["nc"]

    bf16 = ml_dtypes.bfloat16
    x16 = np.ascontiguousarray(inputs, dtype=np.float32).astype(bf16)
    w16 = np.ascontiguousarray(kernel, dtype=np.float32).astype(bf16)
    bias = np.ascontiguousarray(bias, dtype=np.float32)

    in_maps = [
        {
            "x": x16[c * B_CORE : (c + 1) * B_CORE],
            "w": w16,
            "b": bias,
        }
        for c in range(N_CORES)
    ]
    res = run_bass_kernel_spmd(nc, in_maps, list(range(N_CORES)), trace=trace, **kw)
    out = np.concatenate(
        [res.results[c]["y"].astype(np.float32) for c in range(N_CORES)], axis=0
    )
    return out, res


def kernel(**inputs):
    out, _ = _run(inputs["inputs"], inputs["kernel"], inputs["bias"])
    return out


# revision 14
# speedup vs baseline: 1.0253x; 1.0253x over previous
"""Trainium2 Bass kernel for dense layer: out = inputs @ kernel + bias.

Shapes (hardcoded): inputs [16384, 768] f32, kernel [768, 768] f32,
bias [768] f32 -> out [16384, 768] f32.

Strategy: data-parallel over 8 NeuronCores; each core gets a contiguous
2048-row slice of `inputs`, kernel/bias replicated, no collectives.

v2 (bf16 datapath): x and W are rounded to bf16 on the host and y is
returned as bf16 (converted back to f32 on host). Rel err ~3e-3, well
under the 2e-2 gate, and it buys:
  - half the DMA bytes (7.5 MB/core vs 15), so the kernel is purely
    PE-bound and all 16 x tiles are prefetched upfront;
  - 1.0 cyc/row PE transposes (vs 1.5 f32r) and one fused [128,768]
    PSUM->SBUF eviction per tile (vs two);
  - matmul streams at the same 1 col/cycle as f32r, accumulated f32.
Engine layout:
  - sync (SP hwdge): all x-tile loads + W in 3 chunk-group DMAs,
    issued back-to-back upfront in arrival-need order.
  - scalar (Act hwdge): bias load, then all y writebacks -- a second,
    parallel DMA-issue path so y stores never queue behind x loads
    (in v1 a single sync ring serialized 51 DMA issues at ~630ns and
    starved the last tiles).
  - vector: builds the transpose identity on-chip (memset +
    affine_select; no DMA), one CAST eviction + two bias-add
    evictions per tile.
  - gpsimd: bias partition-broadcast only.
  - tensor: 6 transposes + 12 accumulating matmuls per tile; dummy
    identity transposes pad DMA-bound startup stalls so the PE
    p-state/HAM window keeps the clock up.
PSUM: tp 2 + warm 1 + p0 3 + p1 2 = 8 banks.
"""

import sys

for _p in ("/opt/trn_rl_repo", "/root/.axon_site/_ro/trn_rl_repo"):
    if _p not in sys.path:
        sys.path.insert(0, _p)

import numpy as np

B, IN, UNITS = 16384, 768, 768
N_CORES = 8
B_CORE = B // N_CORES          # 2048 rows per core
P = 128
KC = IN // P                   # 6 contraction chunks
NT = B_CORE // P               # 16 row tiles per core
N0, N1 = 512, UNITS - 512      # PSUM bank split of the 768 output cols
NG = 3                         # W arrives in 3 chunk-group DMAs of 2

_cache = {}


def _build_nc():
    import concourse.mybir as mybir
    import concourse.tile as tile
    from concourse import bacc

    f32 = mybir.dt.float32
    bf16 = mybir.dt.bfloat16

    nc = bacc.Bacc()
    x = nc.dram_tensor("x", [B_CORE, IN], bf16, kind="ExternalInput")
    w = nc.dram_tensor("w", [IN, UNITS], bf16, kind="ExternalInput")
    b = nc.dram_tensor("b", [UNITS], f32, kind="ExternalInput")
    y = nc.dram_tensor("y", [B_CORE, UNITS], bf16, kind="ExternalOutput")

    x_v = x.rearrange("(g p) i -> p g i", p=P)   # row tile g, partition p
    y_v = y.rearrange("(g p) u -> p g u", p=P)
    w_v = w.rearrange("(c p) u -> p c u", p=P)   # k-chunk c, partition p

    with tile.TileContext(nc) as tc:
        with (
            tc.tile_pool(name="const", bufs=1) as const,
            tc.tile_pool(name="xin", bufs=NT) as xin,
            tc.tile_pool(name="xt", bufs=4) as xt,
            tc.tile_pool(name="yout", bufs=3) as yout,
            tc.tile_pool(name="tp", bufs=2, space="PSUM") as tp_pool,
            tc.tile_pool(name="pa0", bufs=3, space="PSUM") as pa0_pool,
            tc.tile_pool(name="pa1", bufs=2, space="PSUM") as pa1_pool,
        ):
            # ---- DMA issue: sync = x tiles + W, in arrival-need order ----
            x_bufs = {}

            def dma_x(t):
                xb = xin.tile([P, IN], bf16, tag="x_buf")
                x_bufs[t] = xb
                nc.sync.dma_start(out=xb[:], in_=x_v[:, t, :])

            w_r = const.tile([P, KC, UNITS], bf16, tag="w_r")

            def dma_w(c):
                nc.sync.dma_start(
                    out=w_r[:, c : c + 1, :], in_=w_v[:, c : c + 1, :]
                )

            # single inbound ring (the DMA-fabric ramp is shared, so a
            # second ring only dilutes it): per-chunk W pieces interleaved
            # with x tiles in arrival-need order
            dma_x(0)
            dma_w(0)
            dma_w(1)
            dma_x(1)
            dma_w(2)
            dma_w(3)
            dma_x(2)
            dma_w(4)
            dma_w(5)
            for t in range(3, NT):
                dma_x(t)

            # ---- scalar: bias load (y stores come later on this engine) ----
            bias1 = const.tile([1, UNITS], f32, tag="bias1")
            nc.scalar.dma_start(out=bias1[:], in_=b[None, :])

            # ---- transpose identity built on gpsimd (no DMA): emitted
            # before the bias broadcast so it isn't gated on the bias DMA ----
            ones = const.tile([P, P], bf16, tag="ones")
            nc.gpsimd.memset(ones[:], 1.0)
            ident = const.tile([P, P], bf16, tag="ident")
            nc.gpsimd.affine_select(
                ident[:],
                ones[:],
                pattern=[[1, P]],
                compare_op=mybir.AluOpType.is_equal,
                fill=0.0,
                base=0,
                channel_multiplier=-1,
            )

            bias_b = const.tile([P, UNITS], f32, tag="bias_b")
            nc.gpsimd.partition_broadcast(bias_b[:], bias1[:1, :])

            warm = tp_pool.tile([P, P], bf16, tag="warm", bufs=1)

            def pad(n):
                # dummy PE work: absorbs DMA-bound startup stalls so the
                # PE p-state ramp / HAM window never drops the clock
                for _ in range(n):
                    nc.tensor.transpose(warm[:], ident[:], ident[:])

            def emit_transposes(t):
                """PE-transpose tile t's 6 chunks into one PSUM bank and
                evict with a single fused DVE copy."""
                xn = x_bufs.pop(t)
                tp = tp_pool.tile([P, KC * P], bf16, tag="tp")
                for c in range(KC):
                    nc.tensor.transpose(
                        tp[:, c * P : (c + 1) * P],
                        xn[:, c * P : (c + 1) * P],
                        ident[:],
                    )
                xt_r = xt.tile([P, KC, P], bf16, tag="xt_r")
                nc.vector.tensor_copy(
                    xt_r[:].rearrange("p c b -> p (c b)"), tp[:]
                )
                return xt_r

            def open_accum():
                p0 = pa0_pool.tile([P, N0], f32, tag="p0")
                p1 = pa1_pool.tile([P, N1], f32, tag="p1")
                return p0, p1

            def accum_chunk(xt_r, p0, p1, c):
                lhsT = xt_r[:, c, :]                   # [128 i, 128 b]
                nc.tensor.matmul(
                    p0[:], lhsT, w_r[:, c, 0:N0],
                    start=(c == 0), stop=(c == KC - 1),
                )
                nc.tensor.matmul(
                    p1[:], lhsT, w_r[:, c, N0:UNITS],
                    start=(c == 0), stop=(c == KC - 1),
                )

            def accum_group(xt_r, p0, p1, g):
                for c in (2 * g, 2 * g + 1):
                    accum_chunk(xt_r, p0, p1, c)

            def evict(t, p0, p1):
                # bias-add eviction, per PSUM half; half 1 first so the
                # 2-buf pa1 pool frees a slot earlier (the p1-start matmul
                # two tiles later WAR-waits on this read)
                y_buf = yout.tile([P, UNITS], bf16, tag="y_buf")
                with nc.allow_low_precision(reason="bf16 output rounding"):
                    nc.vector.tensor_add(
                        y_buf[:, N0:UNITS], p1[:], bias_b[:, N0:UNITS]
                    )
                    nc.scalar.dma_start(
                        out=y_v[:, t, N0:UNITS], in_=y_buf[:, N0:UNITS]
                    )
                    nc.vector.tensor_add(
                        y_buf[:, 0:N0], p0[:], bias_b[:, 0:N0]
                    )
                    nc.scalar.dma_start(
                        out=y_v[:, t, 0:N0], in_=y_buf[:, 0:N0]
                    )

            # ---- startup: DMA arrival order is x0, W0, W1, x1, W2, W3,
            # x2, W4, W5, x3, ... — PE work is emitted in exactly that
            # order (PE executes in program order, so a stalled op would
            # block ready work behind it), with pads absorbing the
            # DMA-ramp stalls so the PE p-state keeps climbing ----
            pad(10)
            xts = {0: emit_transposes(0)}
            pad(3)
            pa = {0: open_accum(), 1: open_accum()}
            accum_chunk(xts[0], *pa[0], 0)   # gated on W0
            accum_chunk(xts[0], *pa[0], 1)   # W1
            pad(2)
            xts[1] = emit_transposes(1)      # x1
            accum_chunk(xts[1], *pa[1], 0)
            accum_chunk(xts[1], *pa[1], 1)
            accum_chunk(xts[0], *pa[0], 2)   # W2
            accum_chunk(xts[0], *pa[0], 3)   # W3
            accum_chunk(xts[1], *pa[1], 2)
            accum_chunk(xts[1], *pa[1], 3)
            xts[2] = emit_transposes(2)      # x2
            accum_chunk(xts[0], *pa[0], 4)   # W4
            accum_chunk(xts[0], *pa[0], 5)   # W5
            evict(0, *pa.pop(0))
            accum_chunk(xts[1], *pa[1], 4)
            accum_chunk(xts[1], *pa[1], 5)
            evict(1, *pa.pop(1))
            xts[3] = emit_transposes(3)      # x3

            # ---- steady state: transpose(t+2) right before accum(t), so
            # each tile's CAST eviction lands well before the PE needs it ----
            for t in range(2, NT - 1):
                if t + 2 < NT:
                    xts[t + 2] = emit_transposes(t + 2)
                p0, p1 = open_accum()
                xt_r = xts.pop(t)
                for g in range(NG):
                    accum_group(xt_r, p0, p1, g)
                evict(t, p0, p1)

            # last tile: run all 6 p1 chunks first, then all 6 p0 chunks,
            # so the half-1 eviction + writeback overlap the p0 matmuls
            # and the drain tail shortens
            t = NT - 1
            p0, p1 = open_accum()
            xt_r = xts.pop(t)
            y_buf = yout.tile([P, UNITS], bf16, tag="y_buf")
            for c in range(KC):
                nc.tensor.matmul(
                    p1[:], xt_r[:, c, :], w_r[:, c, N0:UNITS],
                    start=(c == 0), stop=(c == KC - 1),
                )
            with nc.allow_low_precision(reason="bf16 output rounding"):
                nc.vector.tensor_add(
                    y_buf[:, N0:UNITS], p1[:], bias_b[:, N0:UNITS]
                )
            nc.scalar.dma_start(out=y_v[:, t, N0:UNITS], in_=y_buf[:, N0:UNITS])
            for c in range(KC):
                nc.tensor.matmul(
                    p0[:], xt_r[:, c, :], w_r[:, c, 0:N0],
                    start=(c == 0), stop=(c == KC - 1),
                )
            # quarter-split final eviction, issues alternating between the
            # idle sync ring and scalar, to shorten the TT->issue->xfer
            # drain chain after the last matmul
            with nc.allow_low_precision(reason="bf16 output rounding"):
                nc.vector.tensor_add(
                    y_buf[:, 0:256], p0[:, 0:256], bias_b[:, 0:256]
                )
                nc.sync.dma_start(out=y_v[:, t, 0:256], in_=y_buf[:, 0:256])
                nc.vector.tensor_add(
                    y_buf[:, 256:N0], p0[:, 256:N0], bias_b[:, 256:N0]
                )
                nc.scalar.dma_start(
                    out=y_v[:, t, 256:N0], in_=y_buf[:, 256:N0]
                )

    nc.finalize()
    return nc


def _run(inputs, kernel, bias, trace=False, **kw):
    from concourse.bass_utils import run_bass_kernel_spmd
    import ml_dtypes

    if "nc" not in _cache:
        _cache["nc"] = _build_nc()
    nc = _cache["nc"]

    bf16 = ml_dtypes.bfloat16
    x16 = np.ascontiguousarray(inputs, dtype=np.float32).astype(bf16)
    w16 = np.ascontiguousarray(kernel, dtype=np.float32).astype(bf16)
    bias = np.ascontiguousarray(bias, dtype=np.float32)

    in_maps = [
        {
            "x": x16[c * B_CORE : (c + 1) * B_CORE],
            "w": w16,
            "b": bias,
        }
        for c in range(N_CORES)
    ]
    res = run_bass_kernel_spmd(nc, in_maps, list(range(N_CORES)), trace=trace, **kw)
    out = np.concatenate(
        [res.results[c]["y"].astype(np.float32) for c in range(N_CORES)], axis=0
    )
    return out, res


def kernel(**inputs):
    out, _ = _run(inputs["inputs"], inputs["kernel"], inputs["bias"])
    return out


# revision 19
# speedup vs baseline: 1.0357x; 1.0101x over previous
"""Trainium2 Bass kernel for dense layer: out = inputs @ kernel + bias.

Shapes (hardcoded): inputs [16384, 768] f32, kernel [768, 768] f32,
bias [768] f32 -> out [16384, 768] f32.

Strategy: data-parallel over 8 NeuronCores; each core gets a contiguous
2048-row slice of `inputs`, kernel/bias replicated, no collectives.

v2 (bf16 datapath): x and W are rounded to bf16 on the host and y is
returned as bf16 (converted back to f32 on host). Rel err ~3e-3, well
under the 2e-2 gate, and it buys:
  - half the DMA bytes (7.5 MB/core vs 15), so the kernel is purely
    PE-bound and all 16 x tiles are prefetched upfront;
  - 1.0 cyc/row PE transposes (vs 1.5 f32r) and one fused [128,768]
    PSUM->SBUF eviction per tile (vs two);
  - matmul streams at the same 1 col/cycle as f32r, accumulated f32.
Engine layout:
  - sync (SP hwdge): all x-tile loads + W in 3 chunk-group DMAs,
    issued back-to-back upfront in arrival-need order.
  - scalar (Act hwdge): bias load, then all y writebacks -- a second,
    parallel DMA-issue path so y stores never queue behind x loads
    (in v1 a single sync ring serialized 51 DMA issues at ~630ns and
    starved the last tiles).
  - vector: builds the transpose identity on-chip (memset +
    affine_select; no DMA), one CAST eviction + two bias-add
    evictions per tile.
  - gpsimd: bias partition-broadcast only.
  - tensor: 6 transposes + 12 accumulating matmuls per tile; dummy
    identity transposes pad DMA-bound startup stalls so the PE
    p-state/HAM window keeps the clock up.
PSUM: tp 2 + warm 1 + p0 3 + p1 2 = 8 banks.
"""

import sys

for _p in ("/opt/trn_rl_repo", "/root/.axon_site/_ro/trn_rl_repo"):
    if _p not in sys.path:
        sys.path.insert(0, _p)

import numpy as np

B, IN, UNITS = 16384, 768, 768
N_CORES = 8
B_CORE = B // N_CORES          # 2048 rows per core
P = 128
KC = IN // P                   # 6 contraction chunks
NT = B_CORE // P               # 16 row tiles per core
N0, N1 = 512, UNITS - 512      # PSUM bank split of the 768 output cols
NG = 3                         # W arrives in 3 chunk-group DMAs of 2

_cache = {}


def _build_nc():
    import concourse.mybir as mybir
    import concourse.tile as tile
    from concourse import bacc

    f32 = mybir.dt.float32
    bf16 = mybir.dt.bfloat16

    nc = bacc.Bacc()
    x = nc.dram_tensor("x", [B_CORE, IN], bf16, kind="ExternalInput")
    w = nc.dram_tensor("w", [IN, UNITS], bf16, kind="ExternalInput")
    b = nc.dram_tensor("b", [UNITS], f32, kind="ExternalInput")
    y = nc.dram_tensor("y", [B_CORE, UNITS], bf16, kind="ExternalOutput")

    x_v = x.rearrange("(g p) i -> p g i", p=P)   # row tile g, partition p
    y_v = y.rearrange("(g p) u -> p g u", p=P)
    w_v = w.rearrange("(c p) u -> p c u", p=P)   # k-chunk c, partition p

    with tile.TileContext(nc) as tc:
        with (
            tc.tile_pool(name="const", bufs=1) as const,
            tc.tile_pool(name="xin", bufs=NT) as xin,
            tc.tile_pool(name="xt", bufs=4) as xt,
            tc.tile_pool(name="yout", bufs=3) as yout,
            tc.tile_pool(name="tp", bufs=2, space="PSUM") as tp_pool,
            tc.tile_pool(name="pa0", bufs=3, space="PSUM") as pa0_pool,
            tc.tile_pool(name="pa1", bufs=2, space="PSUM") as pa1_pool,
        ):
            # ---- DMA issue: sync = x tiles + W, in arrival-need order ----
            x_bufs = {}

            def dma_x(t):
                xb = xin.tile([P, IN], bf16, tag="x_buf")
                x_bufs[t] = xb
                nc.sync.dma_start(out=xb[:], in_=x_v[:, t, :])

            w_r = const.tile([P, KC, UNITS], bf16, tag="w_r")

            def dma_w(c):
                nc.sync.dma_start(
                    out=w_r[:, c : c + 1, :], in_=w_v[:, c : c + 1, :]
                )

            # single inbound ring (the DMA-fabric ramp is shared, so a
            # second ring only dilutes it): per-chunk W pieces interleaved
            # with x tiles in arrival-need order
            dma_x(0)
            dma_w(0)
            dma_w(1)
            dma_x(1)
            dma_w(2)
            dma_w(3)
            dma_x(2)
            dma_w(4)
            dma_w(5)
            for t in range(3, NT):
                dma_x(t)

            # ---- scalar: bias load (y stores come later on this engine) ----
            bias1 = const.tile([1, UNITS], f32, tag="bias1")
            nc.scalar.dma_start(out=bias1[:], in_=b[None, :])

            # ---- transpose identity built on gpsimd (no DMA): emitted
            # before the bias broadcast so it isn't gated on the bias DMA ----
            ones = const.tile([P, P], bf16, tag="ones")
            nc.gpsimd.memset(ones[:], 1.0)
            ident = const.tile([P, P], bf16, tag="ident")
            nc.gpsimd.affine_select(
                ident[:],
                ones[:],
                pattern=[[1, P]],
                compare_op=mybir.AluOpType.is_equal,
                fill=0.0,
                base=0,
                channel_multiplier=-1,
            )

            bias_b = const.tile([P, UNITS], f32, tag="bias_b")
            nc.gpsimd.partition_broadcast(bias_b[:], bias1[:1, :])

            warm = tp_pool.tile([P, P], bf16, tag="warm", bufs=1)

            def pad(n):
                # dummy PE work: absorbs DMA-bound startup stalls so the
                # PE p-state ramp / HAM window never drops the clock
                for _ in range(n):
                    nc.tensor.transpose(warm[:], ident[:], ident[:])

            def emit_transposes(t):
                """PE-transpose tile t's 6 chunks into one PSUM bank and
                evict with a single fused DVE copy."""
                xn = x_bufs.pop(t)
                tp = tp_pool.tile([P, KC * P], bf16, tag="tp")
                for c in range(KC):
                    nc.tensor.transpose(
                        tp[:, c * P : (c + 1) * P],
                        xn[:, c * P : (c + 1) * P],
                        ident[:],
                    )
                xt_r = xt.tile([P, KC, P], bf16, tag="xt_r")
                nc.vector.tensor_copy(
                    xt_r[:].rearrange("p c b -> p (c b)"), tp[:]
                )
                return xt_r

            def open_accum():
                p0 = pa0_pool.tile([P, N0], f32, tag="p0")
                p1 = pa1_pool.tile([P, N1], f32, tag="p1")
                return p0, p1

            def accum_chunk(xt_r, p0, p1, c):
                lhsT = xt_r[:, c, :]                   # [128 i, 128 b]
                nc.tensor.matmul(
                    p0[:], lhsT, w_r[:, c, 0:N0],
                    start=(c == 0), stop=(c == KC - 1),
                )
                nc.tensor.matmul(
                    p1[:], lhsT, w_r[:, c, N0:UNITS],
                    start=(c == 0), stop=(c == KC - 1),
                )

            def accum_group(xt_r, p0, p1, g):
                for c in (2 * g, 2 * g + 1):
                    accum_chunk(xt_r, p0, p1, c)

            def evict(t, p0, p1):
                # bias-add eviction, per PSUM half; half 1 first so the
                # 2-buf pa1 pool frees a slot earlier (the p1-start matmul
                # two tiles later WAR-waits on this read)
                y_buf = yout.tile([P, UNITS], bf16, tag="y_buf")
                with nc.allow_low_precision(reason="bf16 output rounding"):
                    nc.vector.tensor_add(
                        y_buf[:, N0:UNITS], p1[:], bias_b[:, N0:UNITS]
                    )
                    nc.scalar.dma_start(
                        out=y_v[:, t, N0:UNITS], in_=y_buf[:, N0:UNITS]
                    )
                    nc.vector.tensor_add(
                        y_buf[:, 0:N0], p0[:], bias_b[:, 0:N0]
                    )
                    nc.scalar.dma_start(
                        out=y_v[:, t, 0:N0], in_=y_buf[:, 0:N0]
                    )

            # ---- startup: DMA arrival order is x0, W0, W1, x1, W2, W3,
            # x2, W4, W5, x3, ... — PE work is emitted in exactly that
            # order (PE executes in program order, so a stalled op would
            # block ready work behind it), with pads absorbing the
            # DMA-ramp stalls so the PE p-state keeps climbing ----
            pad(10)
            xts = {0: emit_transposes(0)}
            pad(3)
            pa = {0: open_accum(), 1: open_accum()}
            accum_chunk(xts[0], *pa[0], 0)   # gated on W0
            accum_chunk(xts[0], *pa[0], 1)   # W1
            pad(2)
            xts[1] = emit_transposes(1)      # x1
            accum_chunk(xts[1], *pa[1], 0)
            accum_chunk(xts[1], *pa[1], 1)
            accum_chunk(xts[0], *pa[0], 2)   # W2
            accum_chunk(xts[0], *pa[0], 3)   # W3
            accum_chunk(xts[1], *pa[1], 2)
            accum_chunk(xts[1], *pa[1], 3)
            xts[2] = emit_transposes(2)      # x2
            accum_chunk(xts[0], *pa[0], 4)   # W4
            accum_chunk(xts[0], *pa[0], 5)   # W5
            evict(0, *pa.pop(0))
            accum_chunk(xts[1], *pa[1], 4)
            accum_chunk(xts[1], *pa[1], 5)
            evict(1, *pa.pop(1))
            xts[3] = emit_transposes(3)      # x3

            # ---- steady state: transpose(t+2) right before accum(t), so
            # each tile's CAST eviction lands well before the PE needs it ----
            for t in range(2, NT - 1):
                if t + 2 < NT:
                    xts[t + 2] = emit_transposes(t + 2)
                p0, p1 = open_accum()
                xt_r = xts.pop(t)
                for g in range(NG):
                    accum_group(xt_r, p0, p1, g)
                evict(t, p0, p1)

            # last tile: run all 6 p1 chunks first, then all 6 p0 chunks,
            # so the half-1 eviction + writeback overlap the p0 matmuls
            # and the drain tail shortens
            t = NT - 1
            p0, p1 = open_accum()
            xt_r = xts.pop(t)
            y_buf = yout.tile([P, UNITS], bf16, tag="y_buf")
            for c in range(KC):
                nc.tensor.matmul(
                    p1[:], xt_r[:, c, :], w_r[:, c, N0:UNITS],
                    start=(c == 0), stop=(c == KC - 1),
                )
            with nc.allow_low_precision(reason="bf16 output rounding"):
                nc.vector.tensor_add(
                    y_buf[:, N0:UNITS], p1[:], bias_b[:, N0:UNITS]
                )
            nc.scalar.dma_start(out=y_v[:, t, N0:UNITS], in_=y_buf[:, N0:UNITS])
            for c in range(KC):
                nc.tensor.matmul(
                    p0[:], xt_r[:, c, :], w_r[:, c, 0:N0],
                    start=(c == 0), stop=(c == KC - 1),
                )
            # final eviction's writeback issued on the idle sync ring, so
            # it doesn't queue behind scalar's just-issued half-1 DMA
            with nc.allow_low_precision(reason="bf16 output rounding"):
                nc.vector.tensor_add(y_buf[:, 0:N0], p0[:], bias_b[:, 0:N0])
            nc.sync.dma_start(out=y_v[:, t, 0:N0], in_=y_buf[:, 0:N0])

    nc.finalize()
    return nc


def _run(inputs, kernel, bias, trace=False, **kw):
    from concourse.bass_utils import run_bass_kernel_spmd
    import ml_dtypes

    if "nc" not in _cache:
        _cache["nc"] = _build_nc()
    nc = _cache["nc"]

    bf16 = ml_dtypes.bfloat16
    x16 = np.ascontiguousarray(inputs, dtype=np.float32).astype(bf16)
    w16 = np.ascontiguousarray(kernel, dtype=np.float32).astype(bf16)
    bias = np.ascontiguousarray(bias, dtype=np.float32)

    in_maps = [
        {
            "x": x16[c * B_CORE : (c + 1) * B_CORE],
            "w": w16,
            "b": bias,
        }
        for c in range(N_CORES)
    ]
    res = run_bass_kernel_spmd(nc, in_maps, list(range(N_CORES)), trace=trace, **kw)
    out = np.concatenate(
        [res.results[c]["y"].astype(np.float32) for c in range(N_CORES)], axis=0
    )
    return out, res


def kernel(**inputs):
    out, _ = _run(inputs["inputs"], inputs["kernel"], inputs["bias"])
    return out
